# revision 1
# baseline (speedup 1.0000x reference)
"""Trainium2 Bass kernel for the AdreQwen2 MoE-LoRA SwiGLU MLP.

Problem (hardcoded): B=4, S=2048, H=2048, I=5504, E=8 experts, top-2
per-batch binary gating, rank-16 LoRA adapters on gate/up/down, scale 2.0.

Distribution: token-parallel across 8 NeuronCores (1024 tokens each; each
core's tokens belong to exactly one batch, so its 2 active experts are
fixed). The host pre-selects the top-2 experts per batch and folds the
LoRA adapters into the dense weights exactly (binary gates make this pure
linear algebra): W_eff = W + 2.0 * (A_e0|A_e1 @ B_e0|B_e1)^T. The device
kernel is then a pure dense SwiGLU MLP in bf16 (same 1 col/cycle PE rate
as f32r on TRN2, half the DMA + SBUF; rel err ~3.4e-3 vs the 2e-2 gate).
No collectives: outputs are disjoint token slices, concatenated on host.

Device kernel design (per core, PE floor 4128 matmuls x ~216 ns = 892 us):
 - Fused phases: h = silu(Wg x) * (Wu x) stays RESIDENT in SBUF as bf16
   (86 KB/partition), never spilled to DRAM; phase 2 (Wd h) accumulates
   all 43 K-tiles of an output tile in a single PSUM bank (no SBUF adds).
 - x lives in one resident SBUF tile; weights double-buffer through small
   pools (bf16 streams: Wg+Wu 45 MB, Wd 22.5 MB per core).
 - Each dma_start trigger costs ~0.6 us SERIALLY on the Sync engine, so
   startup issues the fewest triggers in exact consumption order
   (x-chunk0 quarter, wg0 quarter, ...), and steady state uses
   quarter/half-granularity loads paced well under the compute period.
 - 10 warm-up matmuls on a memset tile run during the initial DMA wait so
   the HAM clock gate is at 2.4 GHz when real work arrives; the startup
   critical prefix is 6 serial triggers (x0 ko-quarters + wg0 halves).
 - Consecutive matmuls share their stationary weight slice (both token
   chunks per ko); the final output tile's PSUM->SBUF copy + store is
   split 4-ways to shorten the serial tail.
Measured: ~914 us HW exec (baseline ~972 us f32r spill version), ~97% of
the packed-PE floor.
"""

import sys
import types

import numpy as np

# ---- problem constants (must match setup_inputs) ----
B, S, H, I, E, R = 4, 2048, 2048, 5504, 8, 16
TOP_K = 2
LORA_SCALE = 32.0 / 16.0

P = 128
KH = H // P          # 16 K-tiles over H
KI = I // P          # 43 K-tiles / M-tiles over I
MH = H // P          # 16 M-tiles over H (phase 2 output)
N_CORES = 8
T = B * S            # 8192 tokens
T_CORE = T // N_CORES  # 1024 tokens per core
TCH = 512            # token chunk (matmul moving dim; PSUM bank = 512 f32)
NCHUNK = T_CORE // TCH  # 2

_CACHE: dict = {}


def install_ntff_hook():
    """The antenv stub in this image lacks axon_hooks; reconstruct it so
    run_bass_kernel_spmd(trace=True) can capture NTFF profiles."""
    if "antenv.axon_hooks" in sys.modules:
        return
    try:
        mod = types.ModuleType("antenv.axon_hooks")
        mod._hook = None
        mod.set_axon_ntff_profile_hook = lambda h: setattr(mod, "_hook", h)
        mod.get_axon_ntff_profile_hook = lambda: mod._hook
        sys.modules["antenv.axon_hooks"] = mod
        from trn_agent_boot.trn_boot import _ntff_profile_via_ctypes

        mod.set_axon_ntff_profile_hook(
            _ntff_profile_via_ctypes("/opt/axon/libaxon_pjrt.so")
        )
    except Exception:
        sys.modules.pop("antenv.axon_hooks", None)


def _build_nc():
    import concourse.bacc as bacc
    import concourse.mybir as mybir
    import concourse.tile as tile
    from concourse.bass import ts

    f32 = mybir.dt.float32
    bf16 = mybir.dt.bfloat16
    silu_fn = mybir.ActivationFunctionType.Silu

    nc = bacc.Bacc()

    x_t = nc.declare_dram_parameter("x_t", [P, KH, T_CORE], bf16, isOutput=False)
    wg_t = nc.declare_dram_parameter("wg_t", [KI, P, KH, P], bf16, isOutput=False)
    wu_t = nc.declare_dram_parameter("wu_t", [KI, P, KH, P], bf16, isOutput=False)
    wd_t = nc.declare_dram_parameter("wd_t", [MH, P, KI, P], bf16, isOutput=False)
    outT = nc.declare_dram_parameter("outT", [H, T_CORE], f32, isOutput=True)

    with (
        tile.TileContext(nc) as tc,
        tc.tile_pool(name="xp", bufs=1) as xp,
        tc.tile_pool(name="hp", bufs=1) as hp,
        tc.tile_pool(name="wmp", bufs=1) as wmp,
        tc.tile_pool(name="wgp", bufs=3) as wgp,
        tc.tile_pool(name="wup", bufs=3) as wup,
        tc.tile_pool(name="wdp", bufs=3) as wdp,
        tc.tile_pool(name="work", bufs=3) as work,
        tc.tile_pool(name="outp", bufs=4) as outp,
        tc.tile_pool(name="psg", bufs=2, space="PSUM") as psg,
        tc.tile_pool(name="psup", bufs=2, space="PSUM") as psup,
        tc.tile_pool(name="pso", bufs=4, space="PSUM") as pso,
    ):
        def load_w(pool, tag, src, mi, nko=KH, nsplit=4):
            w_sb = pool.tile([P, nko, P], bf16, tag=tag, name=f"{tag}_{mi}")
            bounds = [nko * q // nsplit for q in range(nsplit + 1)]
            for a, b in zip(bounds, bounds[1:]):
                nc.sync.dma_start(w_sb[:, a:b, :], src[mi][:, a:b, :])
            return w_sb

        # PE warm-up: 8 matmuls on a zero tile, issued before any real work so
        # the HAM clock gate reaches 2.4 GHz while the first x/weight DMAs are
        # still in flight (each dma_start trigger costs ~0.6 us serially on
        # the Sync engine, so the first data lands ~10 us in). PSUM result is
        # never read.
        warm = wmp.tile([P, TCH + P], bf16, tag="warm", name="warm")
        nc.vector.memset(warm[:], 0.0)
        pw = psg.tile([P, TCH], f32, tag="g", name="pg_warm")
        for j in range(10):
            nc.tensor.matmul(
                pw[:],
                warm[:, TCH : TCH + P],
                warm[:, :TCH],
                start=(j == 0),
                stop=(j == 9),
            )

        # x lives in one resident SBUF tile, loaded in 4-ko quarter triggers.
        # Trigger order = consumption order of the first matmul group
        # (x0 quarter, wg0 quarter, ...), minimizing serialized trigger count
        # ahead of the first matmul.
        x_sb = xp.tile([P, KH, T_CORE], bf16, tag="x", name="x_sb")
        wg0 = wgp.tile([P, KH, P], bf16, tag="wg", name="wg_0")
        wu0 = wup.tile([P, KH, P], bf16, tag="wu", name="wu_0")
        # critical prefix is 6 triggers (not 8): x0 stays ko-quarter
        # granular (consumed in ko order), wg0 coarsens to halves
        nc.sync.dma_start(x_sb[:, 0:4, ts(0, TCH)], x_t[:, 0:4, ts(0, TCH)])
        nc.sync.dma_start(wg0[:, 0:8, :], wg_t[0][:, 0:8, :])
        nc.sync.dma_start(x_sb[:, 4:8, ts(0, TCH)], x_t[:, 4:8, ts(0, TCH)])
        nc.sync.dma_start(x_sb[:, 8:12, ts(0, TCH)], x_t[:, 8:12, ts(0, TCH)])
        nc.sync.dma_start(wg0[:, 8:16, :], wg_t[0][:, 8:16, :])
        nc.sync.dma_start(x_sb[:, 12:16, ts(0, TCH)], x_t[:, 12:16, ts(0, TCH)])
        for q in range(4):
            ks = slice(4 * q, 4 * q + 4)
            nc.sync.dma_start(wu0[:, ks, :], wu_t[0][:, ks, :])
        for q in range(4):
            ks = slice(4 * q, 4 * q + 4)
            nc.sync.dma_start(x_sb[:, ks, ts(1, TCH)], x_t[:, ks, ts(1, TCH)])
        x_tiles = [[x_sb[:, ko, ts(n, TCH)] for ko in range(KH)] for n in range(NCHUNK)]

        # ---------------- phase 1: h = silu(Wg x) * (Wu x), resident ----------------
        # Each weight slice feeds both token chunks back-to-back (two PSUM
        # accumulation groups interleaved per ko) so consecutive matmuls share
        # their stationary operand — halves LDWEIGHTS work if codegen dedupes.
        h_tiles = {}
        for mi in range(KI):
            # steady state needs only 4 triggers per mi; the first tiles use
            # quarter loads (already issued for mi 0)
            nsplit = 4 if mi <= 3 else 2
            wg_sb = wg0 if mi == 0 else load_w(wgp, "wg", wg_t, mi, nsplit=nsplit)
            wu_sb = wu0 if mi == 0 else load_w(wup, "wu", wu_t, mi, nsplit=nsplit)
            pg = [psg.tile([P, TCH], f32, tag="g", name=f"pg_{mi}_{n}") for n in range(NCHUNK)]
            pup = [psup.tile([P, TCH], f32, tag="up", name=f"pup_{mi}_{n}") for n in range(NCHUNK)]
            if mi == 0:
                # serial groups: chunk 0 runs while chunk-1 x is still loading
                for mat, psl in ((wg_sb, pg), (wu_sb, pup)):
                    for n in range(NCHUNK):
                        for ko in range(KH):
                            nc.tensor.matmul(
                                psl[n][:],
                                mat[:, ko, :],
                                x_tiles[n][ko],
                                start=(ko == 0),
                                stop=(ko == KH - 1),
                            )
            else:
                # weight-reuse interleaving: both chunks per stationary slice
                for mat, psl in ((wg_sb, pg), (wu_sb, pup)):
                    for ko in range(KH):
                        for n in range(NCHUNK):
                            nc.tensor.matmul(
                                psl[n][:],
                                mat[:, ko, :],
                                x_tiles[n][ko],
                                start=(ko == 0),
                                stop=(ko == KH - 1),
                            )
            for n in range(NCHUNK):
                sil = work.tile([P, TCH], f32, tag="sil", name=f"sil_{mi}_{n}")
                nc.scalar.activation(sil[:], pg[n][:], silu_fn)
                ht = hp.tile([P, TCH], bf16, tag=f"h{n}_{mi}", name=f"h_{n}_{mi}")
                nc.vector.tensor_mul(out=ht[:], in0=sil[:], in1=pup[n][:])
                h_tiles[(n, mi)] = ht

        # ---------------- phase 2: outT = Wd h (PSUM-accumulated) ----------------
        for mh in range(MH):
            wd_sb = load_w(wdp, "wd", wd_t, mh, nko=KI)
            po = [pso.tile([P, TCH], f32, tag="o", name=f"po_{mh}_{n}") for n in range(NCHUNK)]
            if mh < MH - 1:  # weight-reuse interleaving
                for ki in range(KI):
                    for n in range(NCHUNK):
                        nc.tensor.matmul(
                            po[n][:],
                            wd_sb[:, ki, :],
                            h_tiles[(n, ki)][:],
                            start=(ki == 0),
                            stop=(ki == KI - 1),
                        )
            else:  # last mh: serial groups so the n=0 store overlaps n=1 matmuls
                for n in range(NCHUNK):
                    for ki in range(KI):
                        nc.tensor.matmul(
                            po[n][:],
                            wd_sb[:, ki, :],
                            h_tiles[(n, ki)][:],
                            start=(ki == 0),
                            stop=(ki == KI - 1),
                        )
            for n in range(NCHUNK):
                ob = outp.tile([P, TCH], f32, tag="ob", name=f"ob_{mh}_{n}")
                last = mh == MH - 1 and n == NCHUNK - 1
                if not last:
                    nc.scalar.copy(ob[:], po[n][:])
                    nc.sync.dma_start(outT[ts(mh, P), ts(n, TCH)], ob[:])
                else:
                    # shorten the serial tail: pipeline copy->store in
                    # quarters, store triggers alternating across both HWDGE
                    # engines (sync, scalar) to halve trigger serialization
                    QW = TCH // 4
                    for q in range(4):
                        cs = ts(q, QW)
                        nc.scalar.copy(ob[:, cs], po[n][:, cs])
                    for q in range(4):
                        cs = ts(q, QW)
                        (nc.sync, nc.scalar)[q % 2].dma_start(
                            outT[ts(mh, P), n * TCH + q * QW : n * TCH + (q + 1) * QW],
                            ob[:, cs],
                        )

    nc.finalize()
    return nc


def _get_nc():
    if "nc" not in _CACHE:
        _CACHE["nc"] = _build_nc()
    return _CACHE["nc"]


def _tile_kxm(w, n_m, n_k):
    """(M, K) row-major -> (n_m, P, n_k, P) with [mi, p, ko, m] = w[128mi+m, 128ko+p]."""
    return np.ascontiguousarray(w.reshape(n_m, P, n_k, P).transpose(0, 3, 2, 1))


def _prep_inputs(x, gate_values, Wg, Ag, Bg, Wu, Au, Bu, Wd, Ad, Bd):
    """Host-side expert selection, LoRA folding, sharding, and layout prep."""
    import ml_dtypes

    f32 = np.float32
    bf16 = ml_dtypes.bfloat16
    c = np.ascontiguousarray

    xf = np.asarray(x, f32).reshape(T, H)
    gv = np.asarray(gate_values, f32)
    idx = np.argsort(-gv, axis=1)[:, :TOP_K]  # (B, 2) top-2 experts per batch

    Wg_, Wu_, Wd_ = np.asarray(Wg, f32), np.asarray(Wu, f32), np.asarray(Wd, f32)
    Ag_, Bg_ = np.asarray(Ag, f32), np.asarray(Bg, f32)
    Au_, Bu_ = np.asarray(Au, f32), np.asarray(Bu, f32)
    Ad_, Bd_ = np.asarray(Ad, f32), np.asarray(Bd, f32)

    per_batch = []
    for b in range(B):
        es = [int(idx[b, 0]), int(idx[b, 1])]
        # exact LoRA fold: binary top-2 gates => W_eff = W + s * (A_cat @ B_cat)^T
        ag = np.concatenate([Ag_[e] for e in es], axis=1)  # (H, 2R)
        bg = np.concatenate([Bg_[e] for e in es], axis=0)  # (2R, I)
        au = np.concatenate([Au_[e] for e in es], axis=1)
        bu = np.concatenate([Bu_[e] for e in es], axis=0)
        ad = np.concatenate([Ad_[e] for e in es], axis=1)  # (I, 2R)
        bd = np.concatenate([Bd_[e] for e in es], axis=0)  # (2R, H)
        wg_eff = Wg_ + LORA_SCALE * (ag @ bg).T            # (I, H)
        wu_eff = Wu_ + LORA_SCALE * (au @ bu).T            # (I, H)
        wd_eff = Wd_ + LORA_SCALE * (ad @ bd).T            # (H, I)
        per_batch.append(
            (
                _tile_kxm(wg_eff, KI, KH).astype(bf16),
                _tile_kxm(wu_eff, KI, KH).astype(bf16),
                _tile_kxm(wd_eff, MH, KI).astype(bf16),
            )
        )

    in_maps = []
    for core in range(N_CORES):
        b = core * T_CORE // S  # batch this core's tokens belong to
        xc = xf[core * T_CORE : (core + 1) * T_CORE]               # (1024, H)
        x_tl = c(xc.T.reshape(KH, P, T_CORE).transpose(1, 0, 2)).astype(bf16)
        wg_tb, wu_tb, wd_tb = per_batch[b]
        in_maps.append({"x_t": x_tl, "wg_t": wg_tb, "wu_t": wu_tb, "wd_t": wd_tb})
    return in_maps


def _run(inputs, trace=False):
    from concourse.bass_utils import run_bass_kernel_spmd

    if trace:
        install_ntff_hook()
    nc = _get_nc()
    in_maps = _prep_inputs(**inputs)
    res = None
    last_err = None
    for attempt in range(3):  # transient NRT/axon execution errors are retriable
        try:
            res = run_bass_kernel_spmd(
                nc, in_maps, core_ids=list(range(N_CORES)), trace=trace
            )
            break
        except Exception as e:
            last_err = e
    if res is None:
        raise last_err
    outs = [res.results[c]["outT"] for c in range(N_CORES)]  # (H, 1024) each
    full = np.concatenate([o.T for o in outs], axis=0)       # (T, H)
    return full.reshape(B, S, H).astype(np.float32), res


def kernel(**inputs):
    out, _ = _run(inputs, trace=False)
    return out



# revision 5
# speedup vs baseline: 1.0011x; 1.0011x over previous
"""Trainium2 Bass kernel for the AdreQwen2 MoE-LoRA SwiGLU MLP.

Problem (hardcoded): B=4, S=2048, H=2048, I=5504, E=8 experts, top-2
per-batch binary gating, rank-16 LoRA adapters on gate/up/down, scale 2.0.

Distribution: token-parallel across 8 NeuronCores (1024 tokens each; each
core's tokens belong to exactly one batch, so its 2 active experts are
fixed). The host pre-selects the top-2 experts per batch and folds the
LoRA adapters into the dense weights exactly (binary gates make this pure
linear algebra): W_eff = W + 2.0 * (A_e0|A_e1 @ B_e0|B_e1)^T. The device
kernel is then a pure dense SwiGLU MLP in bf16 (same 1 col/cycle PE rate
as f32r on TRN2, half the DMA + SBUF; rel err ~3.4e-3 vs the 2e-2 gate).
No collectives: outputs are disjoint token slices, concatenated on host.

Device kernel design (per core, PE floor 4128 matmuls x ~216 ns = 892 us):
 - Fused phases: h = silu(Wg x) * (Wu x) stays RESIDENT in SBUF as bf16
   (86 KB/partition), never spilled to DRAM; phase 2 (Wd h) accumulates
   all 43 K-tiles of an output tile in a single PSUM bank (no SBUF adds).
 - x lives in one resident SBUF tile; weights double-buffer through small
   pools (bf16 streams: Wg+Wu 45 MB, Wd 22.5 MB per core).
 - Each dma_start trigger costs ~0.6 us SERIALLY on the Sync engine, so
   startup issues the fewest triggers in exact consumption order
   (x-chunk0 quarter, wg0 quarter, ...), and steady state uses
   quarter/half-granularity loads paced well under the compute period.
 - 10 warm-up matmuls on a memset tile run during the initial DMA wait so
   the HAM clock gate is at 2.4 GHz when real work arrives; the startup
   critical prefix is 6 serial triggers (x0 ko-quarters + wg0 halves).
 - Consecutive matmuls share their stationary weight slice (both token
   chunks per ko); the final output tile's PSUM->SBUF copy + store is
   split 4-ways to shorten the serial tail.
Measured: ~914 us HW exec (baseline ~972 us f32r spill version), ~97% of
the packed-PE floor.
"""

import sys
import types

import numpy as np

# ---- problem constants (must match setup_inputs) ----
B, S, H, I, E, R = 4, 2048, 2048, 5504, 8, 16
TOP_K = 2
LORA_SCALE = 32.0 / 16.0

P = 128
KH = H // P          # 16 K-tiles over H
KI = I // P          # 43 K-tiles / M-tiles over I
MH = H // P          # 16 M-tiles over H (phase 2 output)
N_CORES = 8
T = B * S            # 8192 tokens
T_CORE = T // N_CORES  # 1024 tokens per core
TCH = 512            # token chunk (matmul moving dim; PSUM bank = 512 f32)
NCHUNK = T_CORE // TCH  # 2

_CACHE: dict = {}


def install_ntff_hook():
    """The antenv stub in this image lacks axon_hooks; reconstruct it so
    run_bass_kernel_spmd(trace=True) can capture NTFF profiles."""
    if "antenv.axon_hooks" in sys.modules:
        return
    try:
        mod = types.ModuleType("antenv.axon_hooks")
        mod._hook = None
        mod.set_axon_ntff_profile_hook = lambda h: setattr(mod, "_hook", h)
        mod.get_axon_ntff_profile_hook = lambda: mod._hook
        sys.modules["antenv.axon_hooks"] = mod
        from trn_agent_boot.trn_boot import _ntff_profile_via_ctypes

        mod.set_axon_ntff_profile_hook(
            _ntff_profile_via_ctypes("/opt/axon/libaxon_pjrt.so")
        )
    except Exception:
        sys.modules.pop("antenv.axon_hooks", None)


def _build_nc():
    import concourse.bacc as bacc
    import concourse.mybir as mybir
    import concourse.tile as tile
    from concourse.bass import ts

    f32 = mybir.dt.float32
    bf16 = mybir.dt.bfloat16
    silu_fn = mybir.ActivationFunctionType.Silu

    nc = bacc.Bacc()

    x_t = nc.declare_dram_parameter("x_t", [P, KH, T_CORE], bf16, isOutput=False)
    wg_t = nc.declare_dram_parameter("wg_t", [KI, P, KH, P], bf16, isOutput=False)
    wu_t = nc.declare_dram_parameter("wu_t", [KI, P, KH, P], bf16, isOutput=False)
    wd_t = nc.declare_dram_parameter("wd_t", [MH, P, KI, P], bf16, isOutput=False)
    outT = nc.declare_dram_parameter("outT", [H, T_CORE], f32, isOutput=True)

    with (
        tile.TileContext(nc) as tc,
        tc.tile_pool(name="xp", bufs=1) as xp,
        tc.tile_pool(name="hp", bufs=1) as hp,
        tc.tile_pool(name="wmp", bufs=1) as wmp,
        tc.tile_pool(name="wgp", bufs=3) as wgp,
        tc.tile_pool(name="wup", bufs=3) as wup,
        tc.tile_pool(name="wdp", bufs=3) as wdp,
        tc.tile_pool(name="work", bufs=3) as work,
        tc.tile_pool(name="outp", bufs=4) as outp,
        tc.tile_pool(name="psg", bufs=2, space="PSUM") as psg,
        tc.tile_pool(name="psup", bufs=2, space="PSUM") as psup,
        tc.tile_pool(name="pso", bufs=4, space="PSUM") as pso,
    ):
        def load_w(pool, tag, src, mi, nko=KH, nsplit=4):
            w_sb = pool.tile([P, nko, P], bf16, tag=tag, name=f"{tag}_{mi}")
            bounds = [nko * q // nsplit for q in range(nsplit + 1)]
            for a, b in zip(bounds, bounds[1:]):
                nc.sync.dma_start(w_sb[:, a:b, :], src[mi][:, a:b, :])
            return w_sb

        # PE warm-up: small-N matmuls on a tiny zero tile, issued before any
        # real work so the HAM clock gate reaches 2.4 GHz while the first
        # x/weight DMAs are still in flight. The memset is small ([128,256]
        # bf16) so the warm-up starts as soon as the Vector engine is up
        # (~7.3 us) rather than gating on a 640-col memset. PSUM result is
        # never read.
        warm = wmp.tile([P, 2 * P], bf16, tag="warm", name="warm")
        nc.vector.memset(warm[:], 0.0)
        pw = psg.tile([P, TCH], f32, tag="g", name="pg_warm")
        for j in range(24):
            nc.tensor.matmul(
                pw[:, :P],
                warm[:, P : 2 * P],
                warm[:, :P],
                start=(j == 0),
                stop=(j == 23),
            )

        # x lives in one resident SBUF tile. Trigger order = exact consumption
        # order of mi=0's chunk-serial matmul groups (wg-c0, wu-c0, wg-c1,
        # wu-c1): wg0 half, x-c0 ko-pairs interleaved, wu0 halves mid-stream,
        # then x-c1 quads. This keeps the PE DMA-paced (not quarter-granular
        # stalled) through the bandwidth-bound head.
        x_sb = xp.tile([P, KH, T_CORE], bf16, tag="x", name="x_sb")
        wg0 = wgp.tile([P, KH, P], bf16, tag="wg", name="wg_0")
        wu0 = wup.tile([P, KH, P], bf16, tag="wu", name="wu_0")
        nc.sync.dma_start(wg0[:, 0:8, :], wg_t[0][:, 0:8, :])
        nc.sync.dma_start(x_sb[:, 0:2, ts(0, TCH)], x_t[:, 0:2, ts(0, TCH)])
        nc.sync.dma_start(x_sb[:, 2:4, ts(0, TCH)], x_t[:, 2:4, ts(0, TCH)])
        nc.sync.dma_start(wg0[:, 8:16, :], wg_t[0][:, 8:16, :])
        nc.sync.dma_start(x_sb[:, 4:6, ts(0, TCH)], x_t[:, 4:6, ts(0, TCH)])
        nc.sync.dma_start(x_sb[:, 6:8, ts(0, TCH)], x_t[:, 6:8, ts(0, TCH)])
        nc.sync.dma_start(wu0[:, 0:8, :], wu_t[0][:, 0:8, :])
        nc.sync.dma_start(x_sb[:, 8:10, ts(0, TCH)], x_t[:, 8:10, ts(0, TCH)])
        nc.sync.dma_start(x_sb[:, 10:12, ts(0, TCH)], x_t[:, 10:12, ts(0, TCH)])
        nc.sync.dma_start(wu0[:, 8:16, :], wu_t[0][:, 8:16, :])
        nc.sync.dma_start(x_sb[:, 12:14, ts(0, TCH)], x_t[:, 12:14, ts(0, TCH)])
        nc.sync.dma_start(x_sb[:, 14:16, ts(0, TCH)], x_t[:, 14:16, ts(0, TCH)])
        for q in range(4):
            ks = slice(4 * q, 4 * q + 4)
            nc.sync.dma_start(x_sb[:, ks, ts(1, TCH)], x_t[:, ks, ts(1, TCH)])
        x_tiles = [[x_sb[:, ko, ts(n, TCH)] for ko in range(KH)] for n in range(NCHUNK)]

        # ---------------- phase 1: h = silu(Wg x) * (Wu x), resident ----------------
        # Each weight slice feeds both token chunks back-to-back (two PSUM
        # accumulation groups interleaved per ko) so consecutive matmuls share
        # their stationary operand — halves LDWEIGHTS work if codegen dedupes.
        h_tiles = {}
        for mi in range(KI):
            # steady state needs only 4 triggers per mi (halves); the head
            # triggers for mi=0 were already issued above
            wg_sb = wg0 if mi == 0 else load_w(wgp, "wg", wg_t, mi, nsplit=2)
            wu_sb = wu0 if mi == 0 else load_w(wup, "wu", wu_t, mi, nsplit=2)
            pg = [psg.tile([P, TCH], f32, tag="g", name=f"pg_{mi}_{n}") for n in range(NCHUNK)]
            pup = [psup.tile([P, TCH], f32, tag="up", name=f"pup_{mi}_{n}") for n in range(NCHUNK)]
            if mi == 0:
                # chunk-serial groups ordered (wg,c0) (wu,c0) (wg,c1) (wu,c1):
                # chunk-0 work runs while chunk-1 x is still loading, and the
                # x-c1 dependency is deferred to MM #33 instead of #17.
                for n in range(NCHUNK):
                    for mat, psl in ((wg_sb, pg), (wu_sb, pup)):
                        for ko in range(KH):
                            nc.tensor.matmul(
                                psl[n][:],
                                mat[:, ko, :],
                                x_tiles[n][ko],
                                start=(ko == 0),
                                stop=(ko == KH - 1),
                            )
            else:
                # weight-reuse interleaving: both chunks per stationary slice
                for mat, psl in ((wg_sb, pg), (wu_sb, pup)):
                    for ko in range(KH):
                        for n in range(NCHUNK):
                            nc.tensor.matmul(
                                psl[n][:],
                                mat[:, ko, :],
                                x_tiles[n][ko],
                                start=(ko == 0),
                                stop=(ko == KH - 1),
                            )
            for n in range(NCHUNK):
                sil = work.tile([P, TCH], f32, tag="sil", name=f"sil_{mi}_{n}")
                nc.scalar.activation(sil[:], pg[n][:], silu_fn)
                ht = hp.tile([P, TCH], bf16, tag=f"h{n}_{mi}", name=f"h_{n}_{mi}")
                nc.vector.tensor_mul(out=ht[:], in0=sil[:], in1=pup[n][:])
                h_tiles[(n, mi)] = ht

        # ---------------- phase 2: outT = Wd h (PSUM-accumulated) ----------------
        HW = TCH // 2
        for mh in range(MH):
            wd_sb = load_w(wdp, "wd", wd_t, mh, nko=KI)
            if mh < MH - 1:  # weight-reuse interleaving
                po = [pso.tile([P, TCH], f32, tag="o", name=f"po_{mh}_{n}") for n in range(NCHUNK)]
                for ki in range(KI):
                    for n in range(NCHUNK):
                        nc.tensor.matmul(
                            po[n][:],
                            wd_sb[:, ki, :],
                            h_tiles[(n, ki)][:],
                            start=(ki == 0),
                            stop=(ki == KI - 1),
                        )
                for n in range(NCHUNK):
                    ob = outp.tile([P, TCH], f32, tag="ob", name=f"ob_{mh}_{n}")
                    nc.scalar.copy(ob[:], po[n][:])
                    nc.sync.dma_start(outT[ts(mh, P), ts(n, TCH)], ob[:])
            else:
                # Last mh: n=0 runs as one serial group first (its copy+store
                # hide under n=1's matmuls). n=1 is split into two 256-col
                # groups in SEPARATE PSUM banks so the Scalar and Vector
                # engines can drain them in parallel (no same-bank access
                # hazard), with the two stores on different HWDGE queues.
                # This shortens the post-last-matmul chain to one 256-col
                # copy + one store trigger.
                po0 = pso.tile([P, TCH], f32, tag="o", name=f"po_{mh}_0")
                for ki in range(KI):
                    nc.tensor.matmul(
                        po0[:],
                        wd_sb[:, ki, :],
                        h_tiles[(0, ki)][:],
                        start=(ki == 0),
                        stop=(ki == KI - 1),
                    )
                ob0 = outp.tile([P, TCH], f32, tag="ob", name=f"ob_{mh}_0")
                nc.scalar.copy(ob0[:], po0[:])
                nc.sync.dma_start(outT[ts(mh, P), ts(0, TCH)], ob0[:])
                poa = pso.tile([P, TCH], f32, tag="o", name=f"po_{mh}_1a")
                pob = pso.tile([P, TCH], f32, tag="o", name=f"po_{mh}_1b")
                for ki in range(KI):
                    for ph, pt in ((0, poa), (1, pob)):
                        nc.tensor.matmul(
                            pt[:, :HW],
                            wd_sb[:, ki, :],
                            h_tiles[(1, ki)][:, ts(ph, HW)],
                            start=(ki == 0),
                            stop=(ki == KI - 1),
                        )
                oba = outp.tile([P, HW], f32, tag="obq", name=f"ob_{mh}_1a")
                obb = outp.tile([P, HW], f32, tag="obq", name=f"ob_{mh}_1b")
                nc.scalar.copy(oba[:], poa[:, :HW])
                nc.sync.dma_start(outT[ts(mh, P), TCH : TCH + HW], oba[:])
                nc.vector.tensor_scalar_add(obb[:], pob[:, :HW], 0.0)
                nc.scalar.dma_start(outT[ts(mh, P), TCH + HW : 2 * TCH], obb[:])

    nc.finalize()
    return nc


def _get_nc():
    if "nc" not in _CACHE:
        _CACHE["nc"] = _build_nc()
    return _CACHE["nc"]


def _tile_kxm(w, n_m, n_k):
    """(M, K) row-major -> (n_m, P, n_k, P) with [mi, p, ko, m] = w[128mi+m, 128ko+p]."""
    return np.ascontiguousarray(w.reshape(n_m, P, n_k, P).transpose(0, 3, 2, 1))


def _prep_inputs(x, gate_values, Wg, Ag, Bg, Wu, Au, Bu, Wd, Ad, Bd):
    """Host-side expert selection, LoRA folding, sharding, and layout prep."""
    import ml_dtypes

    f32 = np.float32
    bf16 = ml_dtypes.bfloat16
    c = np.ascontiguousarray

    xf = np.asarray(x, f32).reshape(T, H)
    gv = np.asarray(gate_values, f32)
    idx = np.argsort(-gv, axis=1)[:, :TOP_K]  # (B, 2) top-2 experts per batch

    Wg_, Wu_, Wd_ = np.asarray(Wg, f32), np.asarray(Wu, f32), np.asarray(Wd, f32)
    Ag_, Bg_ = np.asarray(Ag, f32), np.asarray(Bg, f32)
    Au_, Bu_ = np.asarray(Au, f32), np.asarray(Bu, f32)
    Ad_, Bd_ = np.asarray(Ad, f32), np.asarray(Bd, f32)

    per_batch = []
    for b in range(B):
        es = [int(idx[b, 0]), int(idx[b, 1])]
        # exact LoRA fold: binary top-2 gates => W_eff = W + s * (A_cat @ B_cat)^T
        ag = np.concatenate([Ag_[e] for e in es], axis=1)  # (H, 2R)
        bg = np.concatenate([Bg_[e] for e in es], axis=0)  # (2R, I)
        au = np.concatenate([Au_[e] for e in es], axis=1)
        bu = np.concatenate([Bu_[e] for e in es], axis=0)
        ad = np.concatenate([Ad_[e] for e in es], axis=1)  # (I, 2R)
        bd = np.concatenate([Bd_[e] for e in es], axis=0)  # (2R, H)
        wg_eff = Wg_ + LORA_SCALE * (ag @ bg).T            # (I, H)
        wu_eff = Wu_ + LORA_SCALE * (au @ bu).T            # (I, H)
        wd_eff = Wd_ + LORA_SCALE * (ad @ bd).T            # (H, I)
        per_batch.append(
            (
                _tile_kxm(wg_eff, KI, KH).astype(bf16),
                _tile_kxm(wu_eff, KI, KH).astype(bf16),
                _tile_kxm(wd_eff, MH, KI).astype(bf16),
            )
        )

    in_maps = []
    for core in range(N_CORES):
        b = core * T_CORE // S  # batch this core's tokens belong to
        xc = xf[core * T_CORE : (core + 1) * T_CORE]               # (1024, H)
        x_tl = c(xc.T.reshape(KH, P, T_CORE).transpose(1, 0, 2)).astype(bf16)
        wg_tb, wu_tb, wd_tb = per_batch[b]
        in_maps.append({"x_t": x_tl, "wg_t": wg_tb, "wu_t": wu_tb, "wd_t": wd_tb})
    return in_maps


def _run(inputs, trace=False):
    from concourse.bass_utils import run_bass_kernel_spmd

    if trace:
        install_ntff_hook()
    nc = _get_nc()
    in_maps = _prep_inputs(**inputs)
    res = None
    last_err = None
    for attempt in range(3):  # transient NRT/axon execution errors are retriable
        try:
            res = run_bass_kernel_spmd(
                nc, in_maps, core_ids=list(range(N_CORES)), trace=trace
            )
            break
        except Exception as e:
            last_err = e
    if res is None:
        raise last_err
    outs = [res.results[c]["outT"] for c in range(N_CORES)]  # (H, 1024) each
    full = np.concatenate([o.T for o in outs], axis=0)       # (T, H)
    return full.reshape(B, S, H).astype(np.float32), res


def kernel(**inputs):
    out, _ = _run(inputs, trace=False)
    return out



# revision 7
# speedup vs baseline: 1.0046x; 1.0034x over previous
"""Trainium2 Bass kernel for the AdreQwen2 MoE-LoRA SwiGLU MLP.

Problem (hardcoded): B=4, S=2048, H=2048, I=5504, E=8 experts, top-2
per-batch binary gating, rank-16 LoRA adapters on gate/up/down, scale 2.0.

Distribution: token-parallel across 8 NeuronCores (1024 tokens each; each
core's tokens belong to exactly one batch, so its 2 active experts are
fixed). The host pre-selects the top-2 experts per batch and folds the
LoRA adapters into the dense weights exactly (binary gates make this pure
linear algebra): W_eff = W + 2.0 * (A_e0|A_e1 @ B_e0|B_e1)^T. The device
kernel is then a pure dense SwiGLU MLP in bf16 (same 1 col/cycle PE rate
as f32r on TRN2, half the DMA + SBUF; rel err ~3.4e-3 vs the 2e-2 gate).
No collectives: outputs are disjoint token slices, concatenated on host.

Device kernel design (per core, PE floor 4128 matmuls x ~216 ns = 892 us):
 - Fused phases: h = silu(Wg x) * (Wu x) stays RESIDENT in SBUF as bf16
   (86 KB/partition), never spilled to DRAM; phase 2 (Wd h) accumulates
   all 43 K-tiles of an output tile in a single PSUM bank (no SBUF adds).
 - x lives in one resident SBUF tile; weights double-buffer through small
   pools (bf16 streams: Wg+Wu 45 MB, Wd 22.5 MB per core).
 - Each dma_start trigger costs ~0.6 us SERIALLY on the Sync engine, so
   startup issues the fewest triggers in exact consumption order
   (x-chunk0 quarter, wg0 quarter, ...), and steady state uses
   quarter/half-granularity loads paced well under the compute period.
 - 10 warm-up matmuls on a memset tile run during the initial DMA wait so
   the HAM clock gate is at 2.4 GHz when real work arrives; the startup
   critical prefix is 6 serial triggers (x0 ko-quarters + wg0 halves).
 - Consecutive matmuls share their stationary weight slice (both token
   chunks per ko); the final output tile's PSUM->SBUF copy + store is
   split 4-ways to shorten the serial tail.
Measured: ~914 us HW exec (baseline ~972 us f32r spill version), ~97% of
the packed-PE floor.
"""

import sys
import types

import numpy as np

# ---- problem constants (must match setup_inputs) ----
B, S, H, I, E, R = 4, 2048, 2048, 5504, 8, 16
TOP_K = 2
LORA_SCALE = 32.0 / 16.0

P = 128
KH = H // P          # 16 K-tiles over H
KI = I // P          # 43 K-tiles / M-tiles over I
MH = H // P          # 16 M-tiles over H (phase 2 output)
N_CORES = 8
T = B * S            # 8192 tokens
T_CORE = T // N_CORES  # 1024 tokens per core
TCH = 512            # token chunk (matmul moving dim; PSUM bank = 512 f32)
NCHUNK = T_CORE // TCH  # 2

_CACHE: dict = {}


def install_ntff_hook():
    """The antenv stub in this image lacks axon_hooks; reconstruct it so
    run_bass_kernel_spmd(trace=True) can capture NTFF profiles."""
    if "antenv.axon_hooks" in sys.modules:
        return
    try:
        mod = types.ModuleType("antenv.axon_hooks")
        mod._hook = None
        mod.set_axon_ntff_profile_hook = lambda h: setattr(mod, "_hook", h)
        mod.get_axon_ntff_profile_hook = lambda: mod._hook
        sys.modules["antenv.axon_hooks"] = mod
        from trn_agent_boot.trn_boot import _ntff_profile_via_ctypes

        mod.set_axon_ntff_profile_hook(
            _ntff_profile_via_ctypes("/opt/axon/libaxon_pjrt.so")
        )
    except Exception:
        sys.modules.pop("antenv.axon_hooks", None)


def _build_nc():
    import concourse.bacc as bacc
    import concourse.mybir as mybir
    import concourse.tile as tile
    from concourse.bass import ts

    f32 = mybir.dt.float32
    bf16 = mybir.dt.bfloat16
    silu_fn = mybir.ActivationFunctionType.Silu

    nc = bacc.Bacc()

    x_t = nc.declare_dram_parameter("x_t", [P, KH, T_CORE], bf16, isOutput=False)
    wg_t = nc.declare_dram_parameter("wg_t", [KI, P, KH, P], bf16, isOutput=False)
    wu_t = nc.declare_dram_parameter("wu_t", [KI, P, KH, P], bf16, isOutput=False)
    wd_t = nc.declare_dram_parameter("wd_t", [MH, P, KI, P], bf16, isOutput=False)
    outT = nc.declare_dram_parameter("outT", [H, T_CORE], f32, isOutput=True)

    with (
        tile.TileContext(nc) as tc,
        tc.tile_pool(name="xp", bufs=1) as xp,
        tc.tile_pool(name="hp", bufs=1) as hp,
        tc.tile_pool(name="wmp", bufs=1) as wmp,
        tc.tile_pool(name="wgp", bufs=3) as wgp,
        tc.tile_pool(name="wup", bufs=3) as wup,
        tc.tile_pool(name="wdp", bufs=3) as wdp,
        tc.tile_pool(name="work", bufs=3) as work,
        tc.tile_pool(name="outp", bufs=4) as outp,
        tc.tile_pool(name="psg", bufs=2, space="PSUM") as psg,
        tc.tile_pool(name="psup", bufs=2, space="PSUM") as psup,
        tc.tile_pool(name="pso", bufs=4, space="PSUM") as pso,
    ):
        def load_w(pool, tag, src, mi, nko=KH, nsplit=4):
            w_sb = pool.tile([P, nko, P], bf16, tag=tag, name=f"{tag}_{mi}")
            bounds = [nko * q // nsplit for q in range(nsplit + 1)]
            for a, b in zip(bounds, bounds[1:]):
                nc.sync.dma_start(w_sb[:, a:b, :], src[mi][:, a:b, :])
            return w_sb

        # PE warm-up: small-N matmuls on a tiny zero tile, issued before any
        # real work so the HAM clock gate reaches 2.4 GHz while the first
        # x/weight DMAs are still in flight. The memset is small ([128,256]
        # bf16) so the warm-up starts as soon as the Vector engine is up
        # (~7.3 us) rather than gating on a 640-col memset. PSUM result is
        # never read.
        warm = wmp.tile([P, 3 * P], bf16, tag="warm", name="warm")
        nc.vector.memset(warm[:], 0.0)
        pw = psg.tile([P, TCH], f32, tag="g", name="pg_warm")
        # 26 N=256 dummies: ~16 cold (213 ns) warm the HAM clock by ~10.6 us,
        # the rest run warm (109 ns) so the PE stays busy until the head DMA
        # stream (wg0 + x ko-pairs) is comfortably ahead (~11.7 us). Real
        # matmuls then run back-to-back without HAM re-throttle.
        NWARM = 26
        for j in range(NWARM):
            nc.tensor.matmul(
                pw[:, : 2 * P],
                warm[:, 2 * P : 3 * P],
                warm[:, : 2 * P],
                start=(j == 0),
                stop=(j == NWARM - 1),
            )

        # x lives in one resident SBUF tile. Trigger order = exact consumption
        # order of mi=0's chunk-serial matmul groups (wg-c0, wu-c0, wg-c1,
        # wu-c1): wg0 half, x-c0 ko-pairs interleaved, wu0 halves mid-stream,
        # then x-c1 quads. This keeps the PE DMA-paced (not quarter-granular
        # stalled) through the bandwidth-bound head.
        x_sb = xp.tile([P, KH, T_CORE], bf16, tag="x", name="x_sb")
        wg0 = wgp.tile([P, KH, P], bf16, tag="wg", name="wg_0")
        wu0 = wup.tile([P, KH, P], bf16, tag="wu", name="wu_0")
        nc.sync.dma_start(wg0[:, 0:8, :], wg_t[0][:, 0:8, :])
        nc.sync.dma_start(x_sb[:, 0:2, ts(0, TCH)], x_t[:, 0:2, ts(0, TCH)])
        nc.sync.dma_start(x_sb[:, 2:4, ts(0, TCH)], x_t[:, 2:4, ts(0, TCH)])
        nc.sync.dma_start(wg0[:, 8:16, :], wg_t[0][:, 8:16, :])
        nc.sync.dma_start(x_sb[:, 4:6, ts(0, TCH)], x_t[:, 4:6, ts(0, TCH)])
        nc.sync.dma_start(x_sb[:, 6:8, ts(0, TCH)], x_t[:, 6:8, ts(0, TCH)])
        nc.sync.dma_start(wu0[:, 0:8, :], wu_t[0][:, 0:8, :])
        nc.sync.dma_start(x_sb[:, 8:10, ts(0, TCH)], x_t[:, 8:10, ts(0, TCH)])
        nc.sync.dma_start(x_sb[:, 10:12, ts(0, TCH)], x_t[:, 10:12, ts(0, TCH)])
        nc.sync.dma_start(wu0[:, 8:16, :], wu_t[0][:, 8:16, :])
        nc.sync.dma_start(x_sb[:, 12:14, ts(0, TCH)], x_t[:, 12:14, ts(0, TCH)])
        nc.sync.dma_start(x_sb[:, 14:16, ts(0, TCH)], x_t[:, 14:16, ts(0, TCH)])
        for q in range(4):
            ks = slice(4 * q, 4 * q + 4)
            nc.sync.dma_start(x_sb[:, ks, ts(1, TCH)], x_t[:, ks, ts(1, TCH)])
        x_tiles = [[x_sb[:, ko, ts(n, TCH)] for ko in range(KH)] for n in range(NCHUNK)]

        # ---------------- phase 1: h = silu(Wg x) * (Wu x), resident ----------------
        # Each weight slice feeds both token chunks back-to-back (two PSUM
        # accumulation groups interleaved per ko) so consecutive matmuls share
        # their stationary operand — halves LDWEIGHTS work if codegen dedupes.
        h_tiles = {}
        for mi in range(KI):
            # steady state needs only 4 triggers per mi (halves); the head
            # triggers for mi=0 were already issued above
            wg_sb = wg0 if mi == 0 else load_w(wgp, "wg", wg_t, mi, nsplit=2)
            wu_sb = wu0 if mi == 0 else load_w(wup, "wu", wu_t, mi, nsplit=2)
            pg = [psg.tile([P, TCH], f32, tag="g", name=f"pg_{mi}_{n}") for n in range(NCHUNK)]
            pup = [psup.tile([P, TCH], f32, tag="up", name=f"pup_{mi}_{n}") for n in range(NCHUNK)]
            if mi == 0:
                # chunk-serial groups ordered (wg,c0) (wu,c0) (wg,c1) (wu,c1):
                # chunk-0 work runs while chunk-1 x is still loading, and the
                # x-c1 dependency is deferred to MM #33 instead of #17.
                for n in range(NCHUNK):
                    if n == 1:
                        # dummy pad absorbing the x-c1 arrival edge so the PE
                        # never idles long enough for a HAM re-throttle
                        for j in range(6):
                            nc.tensor.matmul(
                                pw[:, : 2 * P],
                                warm[:, 2 * P : 3 * P],
                                warm[:, : 2 * P],
                                start=(j == 0),
                                stop=(j == 5),
                            )
                    for mat, psl in ((wg_sb, pg), (wu_sb, pup)):
                        for ko in range(KH):
                            nc.tensor.matmul(
                                psl[n][:],
                                mat[:, ko, :],
                                x_tiles[n][ko],
                                start=(ko == 0),
                                stop=(ko == KH - 1),
                            )
            else:
                # weight-reuse interleaving: both chunks per stationary slice
                for mat, psl in ((wg_sb, pg), (wu_sb, pup)):
                    for ko in range(KH):
                        for n in range(NCHUNK):
                            nc.tensor.matmul(
                                psl[n][:],
                                mat[:, ko, :],
                                x_tiles[n][ko],
                                start=(ko == 0),
                                stop=(ko == KH - 1),
                            )
            for n in range(NCHUNK):
                sil = work.tile([P, TCH], f32, tag="sil", name=f"sil_{mi}_{n}")
                nc.scalar.activation(sil[:], pg[n][:], silu_fn)
                ht = hp.tile([P, TCH], bf16, tag=f"h{n}_{mi}", name=f"h_{n}_{mi}")
                nc.vector.tensor_mul(out=ht[:], in0=sil[:], in1=pup[n][:])
                h_tiles[(n, mi)] = ht

        # ---------------- phase 2: outT = Wd h (PSUM-accumulated) ----------------
        HW = TCH // 2
        for mh in range(MH):
            wd_sb = load_w(wdp, "wd", wd_t, mh, nko=KI)
            if mh < MH - 1:  # weight-reuse interleaving
                po = [pso.tile([P, TCH], f32, tag="o", name=f"po_{mh}_{n}") for n in range(NCHUNK)]
                for ki in range(KI):
                    for n in range(NCHUNK):
                        nc.tensor.matmul(
                            po[n][:],
                            wd_sb[:, ki, :],
                            h_tiles[(n, ki)][:],
                            start=(ki == 0),
                            stop=(ki == KI - 1),
                        )
                for n in range(NCHUNK):
                    ob = outp.tile([P, TCH], f32, tag="ob", name=f"ob_{mh}_{n}")
                    nc.scalar.copy(ob[:], po[n][:])
                    nc.sync.dma_start(outT[ts(mh, P), ts(n, TCH)], ob[:])
            else:
                # Last mh: n=0 runs as one serial group first (its copy+store
                # hide under n=1's matmuls). n=1 is split into two 256-col
                # groups in SEPARATE PSUM banks so the Scalar and Vector
                # engines can drain them in parallel (no same-bank access
                # hazard), with the two stores on different HWDGE queues.
                # This shortens the post-last-matmul chain to one 256-col
                # copy + one store trigger.
                po0 = pso.tile([P, TCH], f32, tag="o", name=f"po_{mh}_0")
                for ki in range(KI):
                    nc.tensor.matmul(
                        po0[:],
                        wd_sb[:, ki, :],
                        h_tiles[(0, ki)][:],
                        start=(ki == 0),
                        stop=(ki == KI - 1),
                    )
                ob0 = outp.tile([P, TCH], f32, tag="ob", name=f"ob_{mh}_0")
                nc.scalar.copy(ob0[:], po0[:])
                nc.sync.dma_start(outT[ts(mh, P), ts(0, TCH)], ob0[:])
                poa = pso.tile([P, TCH], f32, tag="o", name=f"po_{mh}_1a")
                pob = pso.tile([P, TCH], f32, tag="o", name=f"po_{mh}_1b")
                for ki in range(KI):
                    for ph, pt in ((0, poa), (1, pob)):
                        nc.tensor.matmul(
                            pt[:, :HW],
                            wd_sb[:, ki, :],
                            h_tiles[(1, ki)][:, ts(ph, HW)],
                            start=(ki == 0),
                            stop=(ki == KI - 1),
                        )
                oba = outp.tile([P, HW], f32, tag="obq", name=f"ob_{mh}_1a")
                obb = outp.tile([P, HW], f32, tag="obq", name=f"ob_{mh}_1b")
                nc.scalar.copy(oba[:], poa[:, :HW])
                nc.sync.dma_start(outT[ts(mh, P), TCH : TCH + HW], oba[:])
                nc.vector.tensor_scalar_add(obb[:], pob[:, :HW], 0.0)
                nc.scalar.dma_start(outT[ts(mh, P), TCH + HW : 2 * TCH], obb[:])

    nc.finalize()
    return nc


def _get_nc():
    if "nc" not in _CACHE:
        _CACHE["nc"] = _build_nc()
    return _CACHE["nc"]


def _tile_kxm(w, n_m, n_k):
    """(M, K) row-major -> (n_m, P, n_k, P) with [mi, p, ko, m] = w[128mi+m, 128ko+p]."""
    return np.ascontiguousarray(w.reshape(n_m, P, n_k, P).transpose(0, 3, 2, 1))


def _prep_inputs(x, gate_values, Wg, Ag, Bg, Wu, Au, Bu, Wd, Ad, Bd):
    """Host-side expert selection, LoRA folding, sharding, and layout prep."""
    import ml_dtypes

    f32 = np.float32
    bf16 = ml_dtypes.bfloat16
    c = np.ascontiguousarray

    xf = np.asarray(x, f32).reshape(T, H)
    gv = np.asarray(gate_values, f32)
    idx = np.argsort(-gv, axis=1)[:, :TOP_K]  # (B, 2) top-2 experts per batch

    Wg_, Wu_, Wd_ = np.asarray(Wg, f32), np.asarray(Wu, f32), np.asarray(Wd, f32)
    Ag_, Bg_ = np.asarray(Ag, f32), np.asarray(Bg, f32)
    Au_, Bu_ = np.asarray(Au, f32), np.asarray(Bu, f32)
    Ad_, Bd_ = np.asarray(Ad, f32), np.asarray(Bd, f32)

    per_batch = []
    for b in range(B):
        es = [int(idx[b, 0]), int(idx[b, 1])]
        # exact LoRA fold: binary top-2 gates => W_eff = W + s * (A_cat @ B_cat)^T
        ag = np.concatenate([Ag_[e] for e in es], axis=1)  # (H, 2R)
        bg = np.concatenate([Bg_[e] for e in es], axis=0)  # (2R, I)
        au = np.concatenate([Au_[e] for e in es], axis=1)
        bu = np.concatenate([Bu_[e] for e in es], axis=0)
        ad = np.concatenate([Ad_[e] for e in es], axis=1)  # (I, 2R)
        bd = np.concatenate([Bd_[e] for e in es], axis=0)  # (2R, H)
        wg_eff = Wg_ + LORA_SCALE * (ag @ bg).T            # (I, H)
        wu_eff = Wu_ + LORA_SCALE * (au @ bu).T            # (I, H)
        wd_eff = Wd_ + LORA_SCALE * (ad @ bd).T            # (H, I)
        per_batch.append(
            (
                _tile_kxm(wg_eff, KI, KH).astype(bf16),
                _tile_kxm(wu_eff, KI, KH).astype(bf16),
                _tile_kxm(wd_eff, MH, KI).astype(bf16),
            )
        )

    in_maps = []
    for core in range(N_CORES):
        b = core * T_CORE // S  # batch this core's tokens belong to
        xc = xf[core * T_CORE : (core + 1) * T_CORE]               # (1024, H)
        x_tl = c(xc.T.reshape(KH, P, T_CORE).transpose(1, 0, 2)).astype(bf16)
        wg_tb, wu_tb, wd_tb = per_batch[b]
        in_maps.append({"x_t": x_tl, "wg_t": wg_tb, "wu_t": wu_tb, "wd_t": wd_tb})
    return in_maps


def _run(inputs, trace=False):
    from concourse.bass_utils import run_bass_kernel_spmd

    if trace:
        install_ntff_hook()
    nc = _get_nc()
    in_maps = _prep_inputs(**inputs)
    res = None
    last_err = None
    for attempt in range(3):  # transient NRT/axon execution errors are retriable
        try:
            res = run_bass_kernel_spmd(
                nc, in_maps, core_ids=list(range(N_CORES)), trace=trace
            )
            break
        except Exception as e:
            last_err = e
    if res is None:
        raise last_err
    outs = [res.results[c]["outT"] for c in range(N_CORES)]  # (H, 1024) each
    full = np.concatenate([o.T for o in outs], axis=0)       # (T, H)
    return full.reshape(B, S, H).astype(np.float32), res


def kernel(**inputs):
    out, _ = _run(inputs, trace=False)
    return out

